# revision 1
# baseline (speedup 1.0000x reference)
"""Trainium2 Bass kernel for causal multi-head attention (12 heads, S=4096, D=768).

Strategy (8 NeuronCores, SPMD single program):
  - Query-sharded attention with zigzag block assignment: core c owns the four
    128-row query blocks {31-c, 16+c, 15-c, c}, giving every core an equal
    amount of causal work (66 key-blocks total per head).
  - Each core computes Q^T/K^T (transposed) and V (natural, with a ones
    column appended per head for free softmax denominators) for its own rows,
    then one fused AllGather distributes K^T and V to all cores.
  - Attention runs in "scores transposed" orientation [k, q]: softmax needs no
    max subtraction (|score| <= ~8 for this problem, exp is safe in fp32) and
    the exp'd tiles feed the PV matmul directly as the stationary operand with
    zero transposes. The causal diagonal block is masked with an accumulating
    triangular matmul; the denominator is row 64 of the PV output.
  - Per-core work shapes differ (causal), so the attention phase is emitted 8
    times inside tc.If(partition_id == c) branches; projections are uniform.
  - All matmuls run in float32r (TF32-like, ~1.6e-4 rel err, full PE speed at
    N >= 256).
Host side: shards/reorders inputs, runs the SPMD kernel, scatters rows back.
"""

import numpy as np

import concourse.bass as bass
import concourse.tile as tile
from concourse import bacc, mybir
from concourse.bass_utils import run_bass_kernel_spmd

F32 = mybir.dt.float32
F32R = mybir.dt.float32r
BF16 = mybir.dt.bfloat16
AF = mybir.ActivationFunctionType
ALU = mybir.AluOpType

D = 768
NH = 12
DH = 64
S = 4096
NC = 8
QB = 128              # query/key block size
KCH = D // 128        # 6 contraction chunks
NEG = -1e30

# zigzag assignment: core c owns blocks (sorted descending by block index)
TILES = {c: [31 - c, 16 + c, 15 - c, c] for c in range(NC)}

# block g -> (owning rank, index within that rank's sorted tile list)
RANK_OF = {}
IDX_OF = {}
for _r in range(NC):
    for _i, _g in enumerate(TILES[_r]):
        RANK_OF[_g] = _r
        IDX_OF[_g] = _i

# AllGather contribution layout (flat f32):
#   [0, 768*512)          K^T own slice, row-major [768, 512]
#   [KSZ, KSZ + 512*780)  V own slice, row-major [512, 780] = 12 heads x 65
#                         (64 value cols + 1 ones col per head)
KSZ = D * 512
VW = NH * 65          # 780
VSZ = 512 * VW
CSZ = KSZ + VSZ


def _emit_attention(nc, c, cc_out, qt_sb, attn_sb, den_sb, tri_bf, ident_bf,
                    pools):
    """Attention for core c: loads K^T/V from the AllGather output, computes
    scores^T -> exp -> PV (with denominator row) for this core's 4 query
    blocks, writing unnormalized head outputs into attn_sb and denominators
    into den_sb."""
    ktp, vtp, scp, pvp, exp_p = pools
    T = TILES[c]                      # sorted desc
    nblk = 32 - c                     # key blocks needed: g in [0, 31-c]
    diag_slot = {T[s]: s for s in range(4)}

    def nwidth(g):
        return QB * sum(1 for t in T if t >= g)

    for hp in range(6):
        # ---- K^T load: [128 rows = 2 heads x 64, per-rank 512 cols] ----
        kt_sb = ktp.tile([128, NC, 512], F32R, tag="kt")
        for r in range(NC):
            # rank r's sorted blocks [31-r, 16+r, 15-r, r]; skip leading
            # block 31-r when this core never reads it (31-r > 31-c)
            i0 = 1 if r < c else 0
            base = r * CSZ + (hp * 128) * 512
            src = cc_out[base:base + 128 * 512].rearrange(
                "(p f) -> p f", f=512)[:, i0 * 128:512]
            nc.sync.dma_start(out=kt_sb[:, r, i0 * 128:512],
                              in_=src.bitcast(F32R))
        # ---- V loads for the two heads ----
        v_sbs = []
        for p in range(2):
            h = 2 * hp + p
            v_sb = vtp.tile([128, NC, 4, 65], F32R, tag="vt")
            for r in range(NC):
                i0 = 1 if r < c else 0
                base = r * CSZ + KSZ
                src = cc_out[base:base + 512 * VW].rearrange(
                    "(b p f) -> p b f", p=128, f=VW)[:, i0:4, 65 * h:65 * h + 65]
                nc.sync.dma_start(out=v_sb[:, r, i0:4, :],
                                  in_=src.bitcast(F32R))
            v_sbs.append(v_sb)

        pv_ps = [pvp.tile([65, 512], F32, tag="pv", name=f"pv{c}_{hp}_{i}")
                 for i in range(2)]
        qrhs = qt_sb[:, hp, :]

        # iterate key blocks; batch score tiles into 2-bank psum tiles and
        # exp them in large ACT calls
        batch = [None, None]          # per head parity: [psum_tile, used cols]
        pend = [[], []]               # blocks awaiting exp: (g, off, n)

        def flush(p):
            if batch[p] is None or not pend[p]:
                return
            ps_tile, used = batch[p]
            e_sb = exp_p.tile([128, 1024], F32R, tag="exp",
                  name=f"exp{c}_{hp}_{p}_{len(pend[p])}_{pend[p][0][0]}")
            nc.scalar.activation(out=e_sb[:, 0:used], in_=ps_tile[:, 0:used],
                                 func=AF.Exp, scale=1.0)
            for (g, off, n) in pend[p]:
                nc.tensor.matmul(
                    pv_ps[p][:, 0:n],
                    v_sbs[p][:, RANK_OF[g], IDX_OF[g], :],
                    e_sb[:, off:off + n],
                    start=(g == 0), stop=(g == nblk - 1),
                )
            pend[p] = []
            batch[p] = None

        for g in range(nblk):
            n = nwidth(g)
            ko = kt_sb[:, RANK_OF[g], IDX_OF[g] * 128:(IDX_OF[g] + 1) * 128]
            for p in range(2):
                if batch[p] is not None:
                    # a matmul output region must stay within one 512-col
                    # PSUM bank: round up to the next bank when it would cross
                    off = batch[p][1]
                    if off % 512 + n > 512:
                        off = (off + 511) // 512 * 512
                    if off + n > 1024:
                        flush(p)
                    else:
                        batch[p][1] = off
                if batch[p] is None:
                    batch[p] = [scp.tile([128, 1024], F32, tag="sc",
                     name=f"sc{c}_{hp}_{p}_{g}"), 0]
                ps_tile, off = batch[p]
                nc.tensor.matmul(
                    ps_tile[:, off:off + n],
                    ko[64 * p:64 * (p + 1), :],
                    qrhs[64 * p:64 * (p + 1), 0:n],
                    start=True, stop=(g not in diag_slot),
                    tile_position=(64 * p, 0),
                )
                if g in diag_slot:
                    s = diag_slot[g]
                    nc.tensor.matmul(
                        ps_tile[:, off + 128 * s:off + 128 * (s + 1)],
                        tri_bf[:], ident_bf[:],
                        start=False, stop=True,
                    )
                pend[p].append((g, off, n))
                batch[p][1] = off + n
        for p in range(2):
            flush(p)

        # evacuate PV psum: head outputs (unnormalized) + denominator row
        for p in range(2):
            with nc.allow_low_precision(reason="f32r attn intermediate"):
                nc.vector.tensor_copy(attn_sb[64 * p:64 * (p + 1), hp, :],
                                      pv_ps[p][0:64, :])
            nc.vector.tensor_copy(den_sb[0:1, 2 * hp + p, :],
                                  pv_ps[p][64:65, :])


def build_program(debug=False):
    nc = bacc.Bacc("TRN2", target_bir_lowering=False, debug=False,
                   num_devices=NC)

    x_own = nc.dram_tensor('x_own', [512, D], F32, kind='ExternalInput')
    w_qkv = nc.dram_tensor('w_qkv', [D, 3 * D], F32, kind='ExternalInput')
    b_qkv = nc.dram_tensor('b_qkv', [3 * D, 1], F32, kind='ExternalInput')
    w_out = nc.dram_tensor('w_out', [D, D], F32, kind='ExternalInput')
    b_out = nc.dram_tensor('b_out', [1, D], F32, kind='ExternalInput')
    sel12 = nc.dram_tensor('sel12', [NH, D], F32, kind='ExternalInput')
    y = nc.dram_tensor('y', [512, D], F32, kind='ExternalOutput')
    if debug:
        d_qt = nc.dram_tensor('d_qt', [128, 6, 512], F32, kind='ExternalOutput')
        d_cc = nc.dram_tensor('d_cc', [CSZ], F32, kind='ExternalOutput')
        d_att = nc.dram_tensor('d_att', [128, 6, 512], F32, kind='ExternalOutput')
        d_den = nc.dram_tensor('d_den', [NH, 512], F32, kind='ExternalOutput')

    cc_in = nc.dram_tensor('cc_in', [CSZ], F32)
    cc_out = nc.dram_tensor('cc_out', [NC * CSZ], F32, addr_space="Shared")

    with tile.TileContext(nc) as tc:
        with tc.tile_pool(name="const", bufs=1) as const, \
             tc.tile_pool(name="qt", bufs=1) as qtp, \
             tc.tile_pool(name="stg", bufs=3) as stg, \
             tc.tile_pool(name="attn", bufs=1) as attnp, \
             tc.tile_pool(name="io", bufs=2) as iop:

            # ---------------- constants ----------------
            ident_f = const.tile([128, 128], F32)
            nc.gpsimd.memset(ident_f[:], 0.0)
            nc.gpsimd.affine_select(out=ident_f[:], in_=ident_f[:],
                                    compare_op=ALU.not_equal, fill=1.0,
                                    base=0, pattern=[[-1, 128]],
                                    channel_multiplier=1)
            ident_bf = const.tile([128, 128], BF16)
            nc.vector.tensor_copy(ident_bf[:], ident_f[:])
            scr2 = const.tile([128, 128], F32)
            nc.gpsimd.memset(scr2[:], 0.0)
            nc.gpsimd.affine_select(out=scr2[:], in_=scr2[:],
                                    compare_op=ALU.is_ge, fill=NEG,
                                    base=0, pattern=[[-1, 128]],
                                    channel_multiplier=1)
            tri_bf = const.tile([128, 128], BF16)
            nc.vector.tensor_copy(tri_bf[:], scr2[:])
            ones_f = const.tile([1, 128], F32)
            nc.vector.memset(ones_f[:], 1.0)
            ones_r = const.tile([1, 128], F32R)
            nc.vector.tensor_copy(ones_r[:], ones_f[:])
            ones12_f = const.tile([128, 12], F32)
            nc.vector.memset(ones12_f[:], 1.0)

            bq_sb = const.tile([128, 18], F32)
            nc.sync.dma_start(
                out=bq_sb[:],
                in_=b_qkv[:].rearrange("(t p) o -> p (t o)", p=128))
            bv_row = const.tile([1, D], F32R)
            nc.sync.dma_start(
                out=bv_row[:],
                in_=b_qkv[2 * D:3 * D, 0:1].rearrange("a o -> o a").bitcast(F32R))
            bo_row = const.tile([1, D], F32R)
            nc.sync.dma_start(out=bo_row[:], in_=b_out[:].bitcast(F32R))
            sel_sb = const.tile([NH, D], F32R)
            nc.sync.dma_start(out=sel_sb[:], in_=sel12[:].bitcast(F32R))
            wo_sb = const.tile([128, KCH, D], F32R)
            nc.sync.dma_start(
                out=wo_sb[:],
                in_=w_out[:].rearrange("(a p) d -> p a d", p=128).bitcast(F32R))

            qt_sb = qtp.tile([128, 6, 512], F32R)

            # ------------- phase A: projections + AG contribution -------------
            with tc.tile_pool(name="wq", bufs=6) as wqp, \
                 tc.tile_pool(name="xt", bufs=1) as xtp, \
                 tc.tile_pool(name="psA", bufs=2, space="PSUM") as psA:

                x_sb = iop.tile([128, 4, D], F32, tag="xin")
                nc.sync.dma_start(
                    out=x_sb[:],
                    in_=x_own[:].rearrange("(a p) d -> p a d", p=128))
                xt_sb = xtp.tile([128, KCH, 512], F32R)
                for dc in range(KCH):
                    ps_t = psA.tile([128, 512], F32, tag="psA5")
                    for st in range(4):
                        nc.tensor.transpose(
                            ps_t[:, st * 128:(st + 1) * 128],
                            x_sb[:, st, dc * 128:(dc + 1) * 128],
                            ident_f[:])
                    with nc.allow_low_precision(reason="f32r operand"):
                        nc.vector.tensor_copy(xt_sb[:, dc, :], ps_t[:])

                wq_sb = []
                for dc in range(KCH):
                    w_t = wqp.tile([128, 3 * D], F32R, tag="wq")
                    nc.sync.dma_start(
                        out=w_t[:],
                        in_=w_qkv[dc * 128:(dc + 1) * 128, :].bitcast(F32R))
                    wq_sb.append(w_t)

                # qkv^T: Q j-tiles 0..5 on-core; K j-tiles 6..11 -> cc_in
                for jt in range(12):
                    ps_q = psA.tile([128, 512], F32, tag="psA5")
                    for dc in range(KCH):
                        nc.tensor.matmul(
                            ps_q[:],
                            wq_sb[dc][:, jt * 128:(jt + 1) * 128],
                            xt_sb[:, dc, :],
                            start=(dc == 0), stop=(dc == KCH - 1))
                    if jt < 6:
                        with nc.allow_low_precision(reason="f32r q"):
                            nc.vector.tensor_scalar(
                                out=qt_sb[:, jt, :], in0=ps_q[:],
                                scalar1=bq_sb[:, jt:jt + 1], scalar2=0.125,
                                op0=ALU.add, op1=ALU.mult)
                    else:
                        k_st = stg.tile([128, 512], F32, tag="kstage")
                        nc.vector.tensor_scalar(
                            out=k_st[:], in0=ps_q[:],
                            scalar1=bq_sb[:, jt:jt + 1], scalar2=None,
                            op0=ALU.add)
                        kt_base = (jt - 6) * 128 * 512
                        nc.sync.dma_start(
                            out=cc_in[kt_base:kt_base + 128 * 512].rearrange(
                                "(p f) -> p f", f=512),
                            in_=k_st[:])

                # V natural [s, v] with bias via K=1 augmented matmul
                for st in range(4):
                    ps_v = psA.tile([128, D], F32, tag="psAv")
                    for (nb0, nbw) in ((0, 512), (512, 256)):
                        for dc in range(KCH):
                            nc.tensor.matmul(
                                ps_v[:, nb0:nb0 + nbw],
                                xt_sb[:, dc, st * 128:(st + 1) * 128],
                                wq_sb[dc][:, 2 * D + nb0:2 * D + nb0 + nbw],
                                start=(dc == 0), stop=False)
                        nc.tensor.matmul(
                            ps_v[:, nb0:nb0 + nbw],
                            ones_r[:, 0:128],
                            bv_row[:, nb0:nb0 + nbw],
                            start=False, stop=True)
                    v_st = stg.tile([128, VW], F32, tag="vstage")
                    nc.vector.tensor_copy(
                        v_st[:].rearrange("p (h w) -> p h w", w=65)[:, :, 0:64],
                        ps_v[:].rearrange("p (h w) -> p h w", w=64))
                    nc.vector.tensor_copy(
                        v_st[:].rearrange("p (h w) -> p h w", w=65)[:, :, 64:65],
                        ones12_f[:].rearrange("p (h o) -> p h o", o=1))
                    v_base = KSZ + st * 128 * VW
                    nc.sync.dma_start(
                        out=cc_in[v_base:v_base + 128 * VW].rearrange(
                            "(p f) -> p f", f=VW),
                        in_=v_st[:])

            # ---------------- AllGather ----------------
            nc.gpsimd.collective_compute(
                "AllGather", ALU.bypass,
                replica_groups=[list(range(NC))],
                ins=[cc_in[:]],
                outs=[cc_out[:]],
            )

            # ------------- phase B: attention (per-core branches) -------------
            attn_sb = attnp.tile([128, 6, 512], F32R)
            den_sb = attnp.tile([1, NH, 512], F32)
            with tc.tile_pool(name="kt", bufs=2) as ktp, \
                 tc.tile_pool(name="vt", bufs=3) as vtp, \
                 tc.tile_pool(name="exp", bufs=4) as exp_p, \
                 tc.tile_pool(name="sc", bufs=3, space="PSUM") as scp, \
                 tc.tile_pool(name="pv", bufs=2, space="PSUM") as pvp:
                pid = nc.partition_id()
                for c in range(NC):
                    with tc.If(pid == c):
                        _emit_attention(nc, c, cc_out, qt_sb, attn_sb, den_sb,
                                        tri_bf, ident_bf,
                                        (ktp, vtp, scp, pvp, exp_p))

            # ------------- phase C: normalize + out-projection ----------------
            if debug:
                nc.sync.dma_start(out=d_qt[:], in_=qt_sb[:].bitcast(F32))
                nc.sync.dma_start(out=d_cc[:], in_=cc_in[:])
                nc.sync.dma_start(out=d_att[:], in_=attn_sb[:].bitcast(F32))
                nc.sync.dma_start(out=d_den[:],
                                  in_=den_sb[:])
            with tc.tile_pool(name="psC", bufs=2, space="PSUM") as psC:
                den12 = attnp.tile([NH, 512], F32)
                nc.sync.dma_start(
                    out=den12[:],
                    in_=den_sb[:])
                rec_sb = attnp.tile([NH, 512], F32R)
                with nc.allow_low_precision(reason="f32r recip"):
                    nc.vector.reciprocal(rec_sb[:], den12[:])
                for hp in range(6):
                    ps_b = psC.tile([128, 512], F32, tag="ps_bc")
                    nc.tensor.matmul(ps_b[:],
                                     sel_sb[:, hp * 128:(hp + 1) * 128],
                                     rec_sb[:], start=True, stop=True)
                    bc_sb = stg.tile([128, 512], F32R, tag="bcast")
                    with nc.allow_low_precision(reason="f32r bcast"):
                        nc.vector.tensor_copy(bc_sb[:], ps_b[:])
                    with nc.allow_low_precision(reason="f32r normalize"):
                        nc.vector.tensor_tensor(out=attn_sb[:, hp, :],
                                                in0=attn_sb[:, hp, :],
                                                in1=bc_sb[:], op=ALU.mult)

                for qi in range(4):
                    ps_o = psC.tile([128, D], F32, tag="ps_out")
                    for (nb0, nbw) in ((0, 512), (512, 256)):
                        for dc in range(KCH):
                            nc.tensor.matmul(
                                ps_o[:, nb0:nb0 + nbw],
                                attn_sb[:, dc, qi * 128:(qi + 1) * 128],
                                wo_sb[:, dc, nb0:nb0 + nbw],
                                start=(dc == 0), stop=False)
                        nc.tensor.matmul(
                            ps_o[:, nb0:nb0 + nbw],
                            ones_r[:, 0:128],
                            bo_row[:, nb0:nb0 + nbw],
                            start=False, stop=True)
                    y_sb = iop.tile([128, D], F32, tag="yout")
                    nc.vector.tensor_copy(y_sb[:], ps_o[:])
                    nc.sync.dma_start(out=y[qi * 128:(qi + 1) * 128, :],
                                      in_=y_sb[:])

    nc.finalize()
    return nc


_CACHE = {}


def _get_program():
    if 'nc' not in _CACHE:
        _CACHE['nc'] = build_program()
    return _CACHE['nc']


def kernel(x, W_qkv, b_qkv, W_out, b_out, mask):
    x = np.asarray(x, dtype=np.float32)
    W_qkv = np.ascontiguousarray(np.asarray(W_qkv, dtype=np.float32))
    b_qkv = np.asarray(b_qkv, dtype=np.float32)
    W_out = np.ascontiguousarray(np.asarray(W_out, dtype=np.float32))
    b_out = np.asarray(b_out, dtype=np.float32)
    mask = np.asarray(mask)

    causal = np.array_equal(mask[0, 0], np.tril(np.ones((S, S), dtype=bool)))
    if not causal:
        raise NotImplementedError("only causal (tril) mask supported")

    nc = _get_program()

    sel = np.zeros((NH, D), dtype=np.float32)
    for h in range(NH):
        sel[h, h * DH:(h + 1) * DH] = 1.0

    in_maps = []
    for c in range(NC):
        rows = np.concatenate(
            [np.arange(t * QB, (t + 1) * QB) for t in TILES[c]])
        in_maps.append({
            'x_own': np.ascontiguousarray(x[0, rows, :]),
            'w_qkv': W_qkv,
            'b_qkv': np.ascontiguousarray(b_qkv.reshape(3 * D, 1)),
            'w_out': W_out,
            'b_out': np.ascontiguousarray(b_out.reshape(1, D)),
            'sel12': sel,
        })

    res = run_bass_kernel_spmd(nc, in_maps, list(range(NC)))

    out = np.empty((1, S, D), dtype=np.float32)
    for c in range(NC):
        yc = res.results[c]['y']
        for j, t in enumerate(TILES[c]):
            out[0, t * QB:(t + 1) * QB, :] = yc[j * QB:(j + 1) * QB, :]
    return out

